# revision 20
# baseline (speedup 1.0000x reference)
"""Trainium2 Bass kernel for additive-attention pooling.

Math (per batch b):
    h1 = full[b] @ W1 + b1              # [T, U]
    h2 = last[b] @ W2 + b2              # [U]
    score = tanh(h1 + h2) @ V + bV      # [T]   (bV dropped: softmax-invariant)
    attn = softmax_T(score)
    ctx[b] = attn @ full[b]             # [D]

Sharding: data-parallel over B=32 across 8 cores (4 batches each);
params replicated. No collectives.

Per-core dataflow (all matmuls in float32r = full-rate fp32 PE mode):
  - full[b] loaded once, naturally ([t,d] tiles) -> used for the final
    context matmuls (contract t on partitions).
  - fullT ([d,t] tiles) built on-chip with PE transposes (d must sit on
    partitions to contract it in the h1 matmul).
  - h1T[u,t] = W1cols.T @ fullT, accumulated over 4 d-slices in PSUM.
  - tanh + (h2+b1+b2) bias fused in one ScalarE activation that also
    moves PSUM->SBUF (bias is per-partition since u is the partition).
  - score row [1,512] per t-chunk = V_slice.T @ tanh_tile, accumulated
    over 4 u-slices in PSUM.
  - score row -> per-t-tile columns via 16 tiny PE transposes, exp on
    ScalarE, partition-sum via ones-matmul, 1/sum folded into the final
    context scale (so no broadcast of the scalar is ever needed).
"""

import numpy as np

B, T, D, U = 32, 2048, 512, 512
NCORES = 8
BL = B // NCORES  # batches per core
P = 128
DS = D // P   # 4 d-slices
US = U // P   # 4 u-slices
TT = T // P   # 16 t-tiles
NCH = T // 512  # 4 t-chunks of 512

_CACHE = {}


def _build():
    if "nc" in _CACHE:
        return _CACHE["nc"]

    from contextlib import ExitStack

    import concourse.mybir as mybir
    import concourse.tile as tile
    from concourse import bacc
    from concourse.masks import make_identity

    F32 = mybir.dt.float32
    F32R = mybir.dt.float32r
    AF = mybir.ActivationFunctionType

    nc = bacc.Bacc(trn_type="TRN2", target_bir_lowering=False, debug=False)

    full_d = nc.dram_tensor("full", [BL, T, D], F32R, kind="ExternalInput").ap()
    last_d = nc.dram_tensor("last", [BL, D], F32R, kind="ExternalInput").ap()
    w1_d = nc.dram_tensor("W1", [D, U], F32R, kind="ExternalInput").ap()
    b1_d = nc.dram_tensor("b1", [U], F32, kind="ExternalInput").ap()
    w2_d = nc.dram_tensor("W2", [D, U], F32R, kind="ExternalInput").ap()
    b2_d = nc.dram_tensor("b2", [U], F32, kind="ExternalInput").ap()
    v_d = nc.dram_tensor("V", [U, 1], F32R, kind="ExternalInput").ap()
    ctx_d = nc.dram_tensor("ctx", [BL, D], F32, kind="ExternalOutput").ap()

    with tile.TileContext(nc) as tc, ExitStack() as ctx:
        consts = ctx.enter_context(tc.tile_pool(name="consts", bufs=1))
        natp = ctx.enter_context(tc.tile_pool(name="nat", bufs=2))
        ftp = ctx.enter_context(tc.tile_pool(name="ft", bufs=2))
        tanhp = ctx.enter_context(tc.tile_pool(name="tanh", bufs=6))
        smallp = ctx.enter_context(tc.tile_pool(name="small", bufs=2))
        ph1p = ctx.enter_context(tc.tile_pool(name="ph1", bufs=2, space="PSUM"))
        ptrp = ctx.enter_context(tc.tile_pool(name="ptr", bufs=3, space="PSUM"))
        pscp = ctx.enter_context(tc.tile_pool(name="psc", bufs=1, space="PSUM"))
        pmiscp = ctx.enter_context(tc.tile_pool(name="pmisc", bufs=1, space="PSUM"))

        # ---- constants / parameters ----
        ident_f32 = consts.tile([P, P], F32)
        make_identity(nc, ident_f32)
        ident = consts.tile([P, P], F32R)
        nc.vector.tensor_copy(ident, ident_f32)
        ones_f32 = consts.tile([P, 1], F32)
        nc.vector.memset(ones_f32, 1.0)
        ones_col = consts.tile([P, 1], F32R)
        nc.vector.tensor_copy(ones_col, ones_f32)

        w1_sb = consts.tile([P, DS, U], F32R)
        nc.sync.dma_start(w1_sb, w1_d.rearrange("(ds p) u -> p ds u", p=P))
        w2_sb = consts.tile([P, DS, U], F32R)
        nc.sync.dma_start(w2_sb, w2_d.rearrange("(ds p) u -> p ds u", p=P))

        with nc.allow_non_contiguous_dma(reason="small one-off param loads"):
            v_sb = consts.tile([P, US], F32R)
            nc.sync.dma_start(v_sb, v_d.rearrange("(us p) one -> p (us one)", p=P))
            b1_sb = consts.tile([P, US], F32)
            nc.sync.dma_start(b1_sb, b1_d.rearrange("(us p) -> p us", p=P))
            b2_sb = consts.tile([P, US], F32)
            nc.sync.dma_start(b2_sb, b2_d.rearrange("(us p) -> p us", p=P))
            lastT = consts.tile([P, DS, BL], F32R)
            lastT_src = last_d.rearrange("b (ds p) -> p ds b", p=P)
            for ds_ in range(DS):
                nc.sync.dma_start(lastT[:, ds_, :], lastT_src[:, ds_, :])

        # bias[u, b] = h2[b, u] + b1[u] + b2[u]
        b12 = consts.tile([P, US], F32)
        nc.vector.tensor_copy(b12, b1_sb)
        nc.vector.tensor_add(b12, b12, b2_sb)
        bias_sb = consts.tile([P, US, BL], F32)
        for us_ in range(US):
            ph2 = pmiscp.tile([P, 16], F32, tag="pcols")
            for ds_ in range(DS):
                nc.tensor.matmul(
                    ph2[:, :BL],
                    w2_sb[:, ds_, us_ * P:(us_ + 1) * P],
                    lastT[:, ds_, :],
                    start=(ds_ == 0),
                    stop=(ds_ == DS - 1),
                )
            nc.vector.tensor_scalar_add(
                bias_sb[:, us_, :], ph2[:, :BL], b12[:, us_:us_ + 1]
            )

        # ---- per-batch pipeline ----
        for b in range(BL):
            nat = natp.tile([P, TT, D], F32R)
            nat_src = full_d[b].rearrange("(tt p) d -> p tt d", p=P)
            for ch in range(NCH):
                nc.sync.dma_start(
                    nat[:, ch * 4:(ch + 1) * 4, :], nat_src[:, ch * 4:(ch + 1) * 4, :]
                )

            # fullT[d, t] via PE transposes, 4 t-tiles per PSUM bank
            ft = ftp.tile([P, DS, T], F32R)
            for ch in range(NCH):
                for ds_ in range(DS):
                    ptr = ptrp.tile([P, 512], F32R)
                    for k in range(4):
                        tt_ = ch * 4 + k
                        nc.tensor.transpose(
                            ptr[:, k * P:(k + 1) * P],
                            nat[:, tt_, ds_ * P:(ds_ + 1) * P],
                            ident,
                        )
                    nc.vector.tensor_copy(
                        ft[:, ds_, ch * 512:(ch + 1) * 512], ptr
                    )

            # h1T -> tanh(+bias) -> score row chunks
            score_sb = smallp.tile([1, T], F32, tag="scorerow")
            for ch in range(NCH):
                psc = pscp.tile([1, 512], F32)
                for us_ in range(US):
                    ph1 = ph1p.tile([P, 512], F32)
                    for ds_ in range(DS):
                        nc.tensor.matmul(
                            ph1,
                            w1_sb[:, ds_, us_ * P:(us_ + 1) * P],
                            ft[:, ds_, ch * 512:(ch + 1) * 512],
                            start=(ds_ == 0),
                            stop=(ds_ == DS - 1),
                        )
                    th = tanhp.tile([P, 512], F32R)
                    nc.scalar.activation(
                        th, ph1, AF.Tanh, bias=bias_sb[:, us_, b:b + 1]
                    )
                    nc.tensor.matmul(
                        psc,
                        v_sb[:, us_:us_ + 1],
                        th,
                        start=(us_ == 0),
                        stop=(us_ == US - 1),
                    )
                nc.scalar.activation(
                    score_sb[:, ch * 512:(ch + 1) * 512], psc, AF.Copy
                )

            # score row -> columns (t on partitions), exp, sum, 1/sum
            pcols = pmiscp.tile([P, 16], F32, tag="pcols")
            for tt_ in range(TT):
                nc.tensor.transpose(
                    pcols[:, tt_:tt_ + 1],
                    score_sb[:, tt_ * P:(tt_ + 1) * P],
                    ident_f32[0:1, 0:1],
                )
            exp_cols = smallp.tile([P, TT], F32R, tag="expcols")
            nc.scalar.activation(exp_cols, pcols, AF.Exp)

            psum_t = pscp.tile([1, 512], F32, tag="psc")
            nc.tensor.matmul(
                psum_t[:, :TT], ones_col, exp_cols, start=True, stop=True
            )
            sum_sb = smallp.tile([1, 1], F32, tag="sums")
            nc.vector.tensor_reduce(
                sum_sb, psum_t[:, :TT], axis=mybir.AxisListType.X,
                op=mybir.AluOpType.add,
            )
            recip_sb = smallp.tile([1, 1], F32, tag="recip")
            nc.vector.reciprocal(recip_sb, sum_sb)

            # context = (exp_cols.T @ full) / sum
            pctx = pmiscp.tile([1, 512], F32, tag="pctx")
            for tt_ in range(TT):
                nc.tensor.matmul(
                    pctx,
                    exp_cols[:, tt_:tt_ + 1],
                    nat[:, tt_, :],
                    start=(tt_ == 0),
                    stop=(tt_ == TT - 1),
                )
            ctx_row = smallp.tile([1, D], F32, tag="ctxrow")
            nc.vector.tensor_scalar_mul(ctx_row, pctx, recip_sb)
            nc.sync.dma_start(ctx_d[b:b + 1], ctx_row)

    nc.compile()
    _CACHE["nc"] = nc
    return nc


def _runner():
    """Build (once) a cached jitted 8-core executor mirroring
    bass2jax.run_bass_via_pjrt, so repeat calls skip retracing."""
    if "runner" in _CACHE:
        return _CACHE["runner"]

    import jax
    import numpy as _np
    from jax.sharding import Mesh, PartitionSpec
    from jax.experimental.shard_map import shard_map

    import concourse.mybir as mybir
    from concourse import bass2jax

    bass2jax.install_neuronx_cc_hook()
    nc = _build()

    pid_name = nc.partition_id_tensor.name if nc.partition_id_tensor else None
    in_names, out_names, out_avals = [], [], []
    for alloc in nc.m.functions[0].allocations:
        if not isinstance(alloc, mybir.MemoryLocationSet):
            continue
        name = alloc.memorylocations[0].name
        if alloc.kind == "ExternalInput":
            if name != pid_name:
                in_names.append(name)
        elif alloc.kind == "ExternalOutput":
            out_names.append(name)
            out_avals.append(jax.core.ShapedArray(
                tuple(alloc.tensor_shape), mybir.dt.np(alloc.dtype)))
    n_params = len(in_names)
    all_names = in_names + out_names
    if pid_name is not None:
        all_names = all_names + [pid_name]

    def _body(*args):
        operands = list(args)
        if pid_name is not None:
            operands.append(bass2jax.partition_id_tensor())
        outs = bass2jax._bass_exec_p.bind(
            *operands,
            out_avals=tuple(out_avals),
            in_names=tuple(all_names),
            out_names=tuple(out_names),
            lowering_input_output_aliases=(),
            sim_require_finite=True,
            sim_require_nnan=True,
            nc=nc,
        )
        return tuple(outs)

    devices = jax.devices()[:NCORES]
    mesh = Mesh(_np.asarray(devices), ("core",))
    n_outs = len(out_names)
    in_specs = (PartitionSpec("core"),) * (n_params + n_outs)
    out_specs = (PartitionSpec("core"),) * n_outs
    fn = jax.jit(
        shard_map(_body, mesh=mesh, in_specs=in_specs, out_specs=out_specs,
                  check_rep=False),
        keep_unused=True,
    )
    out_zero_shapes = [
        (NCORES * a.shape[0],) + tuple(a.shape[1:]) for a in out_avals
    ]
    _CACHE["runner"] = (fn, in_names, out_names, out_avals, out_zero_shapes)
    return _CACHE["runner"]


def _concat_inputs(full, last, W1, b1, W2, b2, V):
    full = np.ascontiguousarray(np.asarray(full, np.float32))
    last = np.ascontiguousarray(np.asarray(last, np.float32))
    params = {
        "W1": np.ascontiguousarray(np.asarray(W1, np.float32)),
        "b1": np.ascontiguousarray(np.asarray(b1, np.float32)),
        "W2": np.ascontiguousarray(np.asarray(W2, np.float32)),
        "b2": np.ascontiguousarray(np.asarray(b2, np.float32)),
        "V": np.ascontiguousarray(np.asarray(V, np.float32)),
    }
    per_core_data = {"full": full, "last": last}
    _, in_names, _, _, _ = _runner()
    concat = []
    for name in in_names:
        if name in per_core_data:
            concat.append(per_core_data[name])  # axis0 = B = NCORES*BL
        else:
            p = params[name]
            concat.append(np.concatenate([p] * NCORES, axis=0))
    return concat


def kernel(full, last, W1, b1, W2, b2, V, bV, **_unused):
    fn, in_names, out_names, out_avals, out_zero_shapes = _runner()
    concat = _concat_inputs(full, last, W1, b1, W2, b2, V)
    zeros = [np.zeros(s, np.float32) for s in out_zero_shapes]
    outs = fn(*concat, *zeros)
    out = np.asarray(outs[0])  # [B, D]
    return out.astype(np.float32)


def bench(full, last, W1, b1, W2, b2, V, bV=None, iters=20, **_unused):
    """Steady-state per-call time with device-resident inputs (seconds)."""
    import time as _time

    import jax

    fn, in_names, out_names, out_avals, out_zero_shapes = _runner()
    concat = _concat_inputs(full, last, W1, b1, W2, b2, V)
    zeros = [np.zeros(s, np.float32) for s in out_zero_shapes]
    dev_in = [jax.device_put(a) for a in concat]
    dev_zero = [jax.device_put(z) for z in zeros]
    r = fn(*dev_in, *dev_zero)
    jax.block_until_ready(r)
    t0 = _time.time()
    for _ in range(iters):
        r = fn(*dev_in, *dev_zero)
    jax.block_until_ready(r)
    return (_time.time() - t0) / iters


# revision 24
# speedup vs baseline: 1.0040x; 1.0040x over previous
"""Trainium2 Bass kernel for additive-attention pooling.

Math (per batch b):
    h1 = full[b] @ W1 + b1              # [T, U]
    h2 = last[b] @ W2 + b2              # [U]
    score = tanh(h1 + h2) @ V + bV      # [T]   (bV dropped: softmax-invariant)
    attn = softmax_T(score)
    ctx[b] = attn @ full[b]             # [D]

Sharding: data-parallel over B=32 across 8 cores (4 batches each);
params replicated. No collectives.

Per-core dataflow (all matmuls in float32r = full-rate fp32 PE mode):
  - full[b] loaded once, naturally ([t,d] tiles) -> used for the final
    context matmuls (contract t on partitions).
  - fullT ([d,t] tiles) built on-chip with PE transposes (d must sit on
    partitions to contract it in the h1 matmul).
  - h1T[u,t] = W1cols.T @ fullT, accumulated over 4 d-slices in PSUM.
  - tanh + (h2+b1+b2) bias fused in one ScalarE activation that also
    moves PSUM->SBUF (bias is per-partition since u is the partition).
  - score row [1,512] per t-chunk = V_slice.T @ tanh_tile, accumulated
    over 4 u-slices in PSUM.
  - score row -> per-t-tile columns via 16 tiny PE transposes, exp on
    ScalarE, partition-sum via ones-matmul, 1/sum folded into the final
    context scale (so no broadcast of the scalar is ever needed).
"""

import numpy as np

B, T, D, U = 32, 2048, 512, 512
NCORES = 8
BL = B // NCORES  # batches per core
P = 128
DS = D // P   # 4 d-slices
US = U // P   # 4 u-slices
TT = T // P   # 16 t-tiles
NCH = T // 512  # 4 t-chunks of 512

_CACHE = {}


def _build():
    if "nc" in _CACHE:
        return _CACHE["nc"]

    from contextlib import ExitStack

    import concourse.mybir as mybir
    import concourse.tile as tile
    from concourse import bacc
    from concourse.masks import make_identity

    F32 = mybir.dt.float32
    F32R = mybir.dt.float32r
    AF = mybir.ActivationFunctionType

    nc = bacc.Bacc(trn_type="TRN2", target_bir_lowering=False, debug=False)

    full_d = nc.dram_tensor("full", [BL, T, D], F32R, kind="ExternalInput").ap()
    last_d = nc.dram_tensor("last", [BL, D], F32R, kind="ExternalInput").ap()
    w1_d = nc.dram_tensor("W1", [D, U], F32R, kind="ExternalInput").ap()
    b1_d = nc.dram_tensor("b1", [U], F32, kind="ExternalInput").ap()
    w2_d = nc.dram_tensor("W2", [D, U], F32R, kind="ExternalInput").ap()
    b2_d = nc.dram_tensor("b2", [U], F32, kind="ExternalInput").ap()
    v_d = nc.dram_tensor("V", [U, 1], F32R, kind="ExternalInput").ap()
    ctx_d = nc.dram_tensor("ctx", [BL, D], F32, kind="ExternalOutput").ap()

    with tile.TileContext(nc) as tc, ExitStack() as ctx:
        consts = ctx.enter_context(tc.tile_pool(name="consts", bufs=1))
        natp = ctx.enter_context(tc.tile_pool(name="nat", bufs=2))
        ftp = ctx.enter_context(tc.tile_pool(name="ft", bufs=2))
        tanhp = ctx.enter_context(tc.tile_pool(name="tanh", bufs=6))
        smallp = ctx.enter_context(tc.tile_pool(name="small", bufs=2))
        ph1p = ctx.enter_context(tc.tile_pool(name="ph1", bufs=2, space="PSUM"))
        ptrp = ctx.enter_context(tc.tile_pool(name="ptr", bufs=3, space="PSUM"))
        pscp = ctx.enter_context(tc.tile_pool(name="psc", bufs=1, space="PSUM"))
        pmiscp = ctx.enter_context(tc.tile_pool(name="pmisc", bufs=1, space="PSUM"))

        # ---- constants / parameters ----
        ident_f32 = consts.tile([P, P], F32)
        make_identity(nc, ident_f32)
        ident = consts.tile([P, P], F32R)
        nc.vector.tensor_copy(ident, ident_f32)
        ones_f32 = consts.tile([P, 1], F32)
        nc.vector.memset(ones_f32, 1.0)
        ones_col = consts.tile([P, 1], F32R)
        nc.vector.tensor_copy(ones_col, ones_f32)

        w1_sb = consts.tile([P, DS, U], F32R)
        nc.sync.dma_start(w1_sb, w1_d.rearrange("(ds p) u -> p ds u", p=P))
        w2_sb = consts.tile([P, DS, U], F32R)
        nc.sync.dma_start(w2_sb, w2_d.rearrange("(ds p) u -> p ds u", p=P))

        with nc.allow_non_contiguous_dma(reason="small one-off param loads"):
            v_sb = consts.tile([P, US], F32R)
            nc.sync.dma_start(v_sb, v_d.rearrange("(us p) one -> p (us one)", p=P))
            b1_sb = consts.tile([P, US], F32)
            nc.sync.dma_start(b1_sb, b1_d.rearrange("(us p) -> p us", p=P))
            b2_sb = consts.tile([P, US], F32)
            nc.sync.dma_start(b2_sb, b2_d.rearrange("(us p) -> p us", p=P))
            lastT = consts.tile([P, DS, BL], F32R)
            lastT_src = last_d.rearrange("b (ds p) -> p ds b", p=P)
            for ds_ in range(DS):
                nc.sync.dma_start(lastT[:, ds_, :], lastT_src[:, ds_, :])

        # bias[u, b] = h2[b, u] + b1[u] + b2[u]
        b12 = consts.tile([P, US], F32)
        nc.vector.tensor_copy(b12, b1_sb)
        nc.vector.tensor_add(b12, b12, b2_sb)
        bias_sb = consts.tile([P, US, BL], F32)
        for us_ in range(US):
            ph2 = pmiscp.tile([P, 16], F32, tag="pcols")
            for ds_ in range(DS):
                nc.tensor.matmul(
                    ph2[:, :BL],
                    w2_sb[:, ds_, us_ * P:(us_ + 1) * P],
                    lastT[:, ds_, :],
                    start=(ds_ == 0),
                    stop=(ds_ == DS - 1),
                )
            nc.vector.tensor_scalar_add(
                bias_sb[:, us_, :], ph2[:, :BL], b12[:, us_:us_ + 1]
            )

        # ---- per-batch pipeline ----
        for b in range(BL):
            nat = natp.tile([P, TT, D], F32R)
            nat_src = full_d[b].rearrange("(tt p) d -> p tt d", p=P)
            if b == 0:
                # finer first loads so the first transposes start sooner
                for tt_ in range(4):
                    nc.sync.dma_start(
                        nat[:, tt_:tt_ + 1, :], nat_src[:, tt_:tt_ + 1, :]
                    )
                for ch in range(1, NCH):
                    nc.sync.dma_start(
                        nat[:, ch * 4:(ch + 1) * 4, :],
                        nat_src[:, ch * 4:(ch + 1) * 4, :],
                    )
            else:
                for ch in range(NCH):
                    nc.sync.dma_start(
                        nat[:, ch * 4:(ch + 1) * 4, :],
                        nat_src[:, ch * 4:(ch + 1) * 4, :],
                    )

            # fullT[d, t] via PE transposes, 4 t-tiles per PSUM bank
            ft = ftp.tile([P, DS, T], F32R)
            for ch in range(NCH):
                for ds_ in range(DS):
                    ptr = ptrp.tile([P, 512], F32R)
                    for k in range(4):
                        tt_ = ch * 4 + k
                        nc.tensor.transpose(
                            ptr[:, k * P:(k + 1) * P],
                            nat[:, tt_, ds_ * P:(ds_ + 1) * P],
                            ident,
                        )
                    nc.vector.tensor_copy(
                        ft[:, ds_, ch * 512:(ch + 1) * 512], ptr
                    )

            # h1T -> tanh(+bias) -> score row chunks
            score_sb = smallp.tile([1, T], F32, tag="scorerow")
            for ch in range(NCH):
                psc = pscp.tile([1, 512], F32)
                for us_ in range(US):
                    ph1 = ph1p.tile([P, 512], F32)
                    for ds_ in range(DS):
                        nc.tensor.matmul(
                            ph1,
                            w1_sb[:, ds_, us_ * P:(us_ + 1) * P],
                            ft[:, ds_, ch * 512:(ch + 1) * 512],
                            start=(ds_ == 0),
                            stop=(ds_ == DS - 1),
                        )
                    th = tanhp.tile([P, 512], F32R)
                    nc.scalar.activation(
                        th, ph1, AF.Tanh, bias=bias_sb[:, us_, b:b + 1]
                    )
                    nc.tensor.matmul(
                        psc,
                        v_sb[:, us_:us_ + 1],
                        th,
                        start=(us_ == 0),
                        stop=(us_ == US - 1),
                    )
                nc.scalar.activation(
                    score_sb[:, ch * 512:(ch + 1) * 512], psc, AF.Copy
                )

            # score row -> columns (t on partitions), exp, sum, 1/sum
            pcols = pmiscp.tile([P, 16], F32, tag="pcols")
            for tt_ in range(TT):
                nc.tensor.transpose(
                    pcols[:, tt_:tt_ + 1],
                    score_sb[:, tt_ * P:(tt_ + 1) * P],
                    ident_f32[0:1, 0:1],
                )
            exp_cols = smallp.tile([P, TT], F32R, tag="expcols")
            nc.scalar.activation(exp_cols, pcols, AF.Exp)

            psum_t = pscp.tile([1, 512], F32, tag="psc")
            nc.tensor.matmul(
                psum_t[:, :TT], ones_col, exp_cols, start=True, stop=True
            )
            sum_sb = smallp.tile([1, 1], F32, tag="sums")
            nc.vector.tensor_reduce(
                sum_sb, psum_t[:, :TT], axis=mybir.AxisListType.X,
                op=mybir.AluOpType.add,
            )
            recip_sb = smallp.tile([1, 1], F32, tag="recip")
            nc.vector.reciprocal(recip_sb, sum_sb)

            # context = (exp_cols.T @ full) / sum
            pctx = pmiscp.tile([1, 512], F32, tag="pctx")
            for tt_ in range(TT):
                nc.tensor.matmul(
                    pctx,
                    exp_cols[:, tt_:tt_ + 1],
                    nat[:, tt_, :],
                    start=(tt_ == 0),
                    stop=(tt_ == TT - 1),
                )
            ctx_row = smallp.tile([1, D], F32, tag="ctxrow")
            nc.vector.tensor_scalar_mul(ctx_row, pctx, recip_sb)
            nc.sync.dma_start(ctx_d[b:b + 1], ctx_row)

    nc.compile()
    _CACHE["nc"] = nc
    return nc


def _runner():
    """Build (once) a cached jitted 8-core executor mirroring
    bass2jax.run_bass_via_pjrt, so repeat calls skip retracing."""
    if "runner" in _CACHE:
        return _CACHE["runner"]

    import jax
    import numpy as _np
    from jax.sharding import Mesh, PartitionSpec
    from jax.experimental.shard_map import shard_map

    import concourse.mybir as mybir
    from concourse import bass2jax

    bass2jax.install_neuronx_cc_hook()
    nc = _build()

    pid_name = nc.partition_id_tensor.name if nc.partition_id_tensor else None
    in_names, out_names, out_avals = [], [], []
    for alloc in nc.m.functions[0].allocations:
        if not isinstance(alloc, mybir.MemoryLocationSet):
            continue
        name = alloc.memorylocations[0].name
        if alloc.kind == "ExternalInput":
            if name != pid_name:
                in_names.append(name)
        elif alloc.kind == "ExternalOutput":
            out_names.append(name)
            out_avals.append(jax.core.ShapedArray(
                tuple(alloc.tensor_shape), mybir.dt.np(alloc.dtype)))
    n_params = len(in_names)
    all_names = in_names + out_names
    if pid_name is not None:
        all_names = all_names + [pid_name]

    def _body(*args):
        operands = list(args)
        if pid_name is not None:
            operands.append(bass2jax.partition_id_tensor())
        outs = bass2jax._bass_exec_p.bind(
            *operands,
            out_avals=tuple(out_avals),
            in_names=tuple(all_names),
            out_names=tuple(out_names),
            lowering_input_output_aliases=(),
            sim_require_finite=True,
            sim_require_nnan=True,
            nc=nc,
        )
        return tuple(outs)

    devices = jax.devices()[:NCORES]
    mesh = Mesh(_np.asarray(devices), ("core",))
    n_outs = len(out_names)
    in_specs = (PartitionSpec("core"),) * (n_params + n_outs)
    out_specs = (PartitionSpec("core"),) * n_outs
    fn = jax.jit(
        shard_map(_body, mesh=mesh, in_specs=in_specs, out_specs=out_specs,
                  check_rep=False),
        keep_unused=True,
    )
    out_zero_shapes = [
        (NCORES * a.shape[0],) + tuple(a.shape[1:]) for a in out_avals
    ]
    _CACHE["runner"] = (fn, in_names, out_names, out_avals, out_zero_shapes)
    return _CACHE["runner"]


def _concat_inputs(full, last, W1, b1, W2, b2, V):
    full = np.ascontiguousarray(np.asarray(full, np.float32))
    last = np.ascontiguousarray(np.asarray(last, np.float32))
    params = {
        "W1": np.ascontiguousarray(np.asarray(W1, np.float32)),
        "b1": np.ascontiguousarray(np.asarray(b1, np.float32)),
        "W2": np.ascontiguousarray(np.asarray(W2, np.float32)),
        "b2": np.ascontiguousarray(np.asarray(b2, np.float32)),
        "V": np.ascontiguousarray(np.asarray(V, np.float32)),
    }
    per_core_data = {"full": full, "last": last}
    _, in_names, _, _, _ = _runner()
    concat = []
    for name in in_names:
        if name in per_core_data:
            concat.append(per_core_data[name])  # axis0 = B = NCORES*BL
        else:
            p = params[name]
            concat.append(np.concatenate([p] * NCORES, axis=0))
    return concat


def kernel(full, last, W1, b1, W2, b2, V, bV, **_unused):
    fn, in_names, out_names, out_avals, out_zero_shapes = _runner()
    concat = _concat_inputs(full, last, W1, b1, W2, b2, V)
    zeros = [np.zeros(s, np.float32) for s in out_zero_shapes]
    outs = fn(*concat, *zeros)
    out = np.asarray(outs[0])  # [B, D]
    return out.astype(np.float32)


def bench(full, last, W1, b1, W2, b2, V, bV=None, iters=20, **_unused):
    """Steady-state per-call time with device-resident inputs (seconds)."""
    import time as _time

    import jax

    fn, in_names, out_names, out_avals, out_zero_shapes = _runner()
    concat = _concat_inputs(full, last, W1, b1, W2, b2, V)
    zeros = [np.zeros(s, np.float32) for s in out_zero_shapes]
    dev_in = [jax.device_put(a) for a in concat]
    dev_zero = [jax.device_put(z) for z in zeros]
    r = fn(*dev_in, *dev_zero)
    jax.block_until_ready(r)
    t0 = _time.time()
    for _ in range(iters):
        r = fn(*dev_in, *dev_zero)
    jax.block_until_ready(r)
    return (_time.time() - t0) / iters


# revision 27
# speedup vs baseline: 1.0087x; 1.0047x over previous
"""Trainium2 Bass kernel for additive-attention pooling.

Math (per batch b):
    h1 = full[b] @ W1 + b1              # [T, U]
    h2 = last[b] @ W2 + b2              # [U]
    score = tanh(h1 + h2) @ V + bV      # [T]   (bV dropped: softmax-invariant)
    attn = softmax_T(score)
    ctx[b] = attn @ full[b]             # [D]

Sharding: data-parallel over B=32 across 8 cores (4 batches each);
params replicated. No collectives.

Per-core dataflow (all matmuls in float32r = full-rate fp32 PE mode):
  - full[b] loaded once, naturally ([t,d] tiles) -> used for the final
    context matmuls (contract t on partitions).
  - fullT ([d,t] tiles) built on-chip with PE transposes (d must sit on
    partitions to contract it in the h1 matmul).
  - h1T[u,t] = W1cols.T @ fullT, accumulated over 4 d-slices in PSUM.
  - tanh + (h2+b1+b2) bias fused in one ScalarE activation that also
    moves PSUM->SBUF (bias is per-partition since u is the partition).
  - score row [1,512] per t-chunk = V_slice.T @ tanh_tile, accumulated
    over 4 u-slices in PSUM.
  - score row -> per-t-tile columns via 16 tiny PE transposes, exp on
    ScalarE, partition-sum via ones-matmul, 1/sum folded into the final
    context scale (so no broadcast of the scalar is ever needed).
"""

import numpy as np

B, T, D, U = 32, 2048, 512, 512
NCORES = 8
BL = B // NCORES  # batches per core
P = 128
DS = D // P   # 4 d-slices
US = U // P   # 4 u-slices
TT = T // P   # 16 t-tiles
NCH = T // 512  # 4 t-chunks of 512

_CACHE = {}


def _build():
    if "nc" in _CACHE:
        return _CACHE["nc"]

    from contextlib import ExitStack

    import concourse.mybir as mybir
    import concourse.tile as tile
    from concourse import bacc
    from concourse.masks import make_identity

    F32 = mybir.dt.float32
    F32R = mybir.dt.float32r
    AF = mybir.ActivationFunctionType

    nc = bacc.Bacc(trn_type="TRN2", target_bir_lowering=False, debug=False)

    full_d = nc.dram_tensor("full", [BL, T, D], F32R, kind="ExternalInput").ap()
    last_d = nc.dram_tensor("last", [BL, D], F32R, kind="ExternalInput").ap()
    w1_d = nc.dram_tensor("W1", [D, U], F32R, kind="ExternalInput").ap()
    b1_d = nc.dram_tensor("b1", [U], F32, kind="ExternalInput").ap()
    w2_d = nc.dram_tensor("W2", [D, U], F32R, kind="ExternalInput").ap()
    b2_d = nc.dram_tensor("b2", [U], F32, kind="ExternalInput").ap()
    v_d = nc.dram_tensor("V", [U, 1], F32R, kind="ExternalInput").ap()
    ctx_d = nc.dram_tensor("ctx", [BL, D], F32, kind="ExternalOutput").ap()

    with tile.TileContext(nc) as tc, ExitStack() as ctx:
        consts = ctx.enter_context(tc.tile_pool(name="consts", bufs=1))
        natp = ctx.enter_context(tc.tile_pool(name="nat", bufs=2))
        ftp = ctx.enter_context(tc.tile_pool(name="ft", bufs=2))
        tanhp = ctx.enter_context(tc.tile_pool(name="tanh", bufs=6))
        smallp = ctx.enter_context(tc.tile_pool(name="small", bufs=2))
        ph1p = ctx.enter_context(tc.tile_pool(name="ph1", bufs=2, space="PSUM"))
        ptrp = ctx.enter_context(tc.tile_pool(name="ptr", bufs=3, space="PSUM"))
        pscp = ctx.enter_context(tc.tile_pool(name="psc", bufs=1, space="PSUM"))
        pmiscp = ctx.enter_context(tc.tile_pool(name="pmisc", bufs=1, space="PSUM"))

        # ---- constants / parameters ----
        ident_f32 = consts.tile([P, P], F32)
        make_identity(nc, ident_f32)
        ident = consts.tile([P, P], F32R)
        nc.vector.tensor_copy(ident, ident_f32)
        ones_f32 = consts.tile([P, 1], F32)
        nc.vector.memset(ones_f32, 1.0)
        ones_col = consts.tile([P, 1], F32R)
        nc.vector.tensor_copy(ones_col, ones_f32)

        w1_sb = consts.tile([P, DS, U], F32R)
        nc.sync.dma_start(w1_sb, w1_d.rearrange("(ds p) u -> p ds u", p=P))
        w2_sb = consts.tile([P, DS, U], F32R)
        nc.sync.dma_start(w2_sb, w2_d.rearrange("(ds p) u -> p ds u", p=P))

        with nc.allow_non_contiguous_dma(reason="small one-off param loads"):
            v_sb = consts.tile([P, US], F32R)
            nc.sync.dma_start(v_sb, v_d.rearrange("(us p) one -> p (us one)", p=P))
            b1_sb = consts.tile([P, US], F32)
            nc.sync.dma_start(b1_sb, b1_d.rearrange("(us p) -> p us", p=P))
            b2_sb = consts.tile([P, US], F32)
            nc.sync.dma_start(b2_sb, b2_d.rearrange("(us p) -> p us", p=P))
            lastT = consts.tile([P, DS, BL], F32R)
            lastT_src = last_d.rearrange("b (ds p) -> p ds b", p=P)
            for ds_ in range(DS):
                nc.sync.dma_start(lastT[:, ds_, :], lastT_src[:, ds_, :])

        # bias[u, b] = h2[b, u] + b1[u] + b2[u]
        b12 = consts.tile([P, US], F32)
        nc.vector.tensor_copy(b12, b1_sb)
        nc.vector.tensor_add(b12, b12, b2_sb)
        bias_sb = consts.tile([P, US, BL], F32)
        for us_ in range(US):
            ph2 = pmiscp.tile([P, 16], F32, tag="pcols")
            for ds_ in range(DS):
                nc.tensor.matmul(
                    ph2[:, :BL],
                    w2_sb[:, ds_, us_ * P:(us_ + 1) * P],
                    lastT[:, ds_, :],
                    start=(ds_ == 0),
                    stop=(ds_ == DS - 1),
                )
            nc.vector.tensor_scalar_add(
                bias_sb[:, us_, :], ph2[:, :BL], b12[:, us_:us_ + 1]
            )

        # ---- per-batch pipeline ----
        for b in range(BL):
            nat = natp.tile([P, TT, D], F32R)
            nat_src = full_d[b].rearrange("(tt p) d -> p tt d", p=P)
            if b == 0:
                # d-slab first loads: transpose group (ch0, ds) needs only
                # slab ds of the first 4 t-tiles (256KB), so PE starts sooner
                for ds_ in range(DS):
                    nc.sync.dma_start(
                        nat[:, 0:4, ds_ * P:(ds_ + 1) * P],
                        nat_src[:, 0:4, ds_ * P:(ds_ + 1) * P],
                    )
                for ch in range(1, NCH):
                    nc.sync.dma_start(
                        nat[:, ch * 4:(ch + 1) * 4, :],
                        nat_src[:, ch * 4:(ch + 1) * 4, :],
                    )
            else:
                for ch in range(NCH):
                    nc.sync.dma_start(
                        nat[:, ch * 4:(ch + 1) * 4, :],
                        nat_src[:, ch * 4:(ch + 1) * 4, :],
                    )

            # fullT[d, t] via PE transposes, 4 t-tiles per PSUM bank
            ft = ftp.tile([P, DS, T], F32R)
            for ch in range(NCH):
                for ds_ in range(DS):
                    ptr = ptrp.tile([P, 512], F32R)
                    for k in range(4):
                        tt_ = ch * 4 + k
                        nc.tensor.transpose(
                            ptr[:, k * P:(k + 1) * P],
                            nat[:, tt_, ds_ * P:(ds_ + 1) * P],
                            ident,
                        )
                    nc.vector.tensor_copy(
                        ft[:, ds_, ch * 512:(ch + 1) * 512], ptr
                    )

            # h1T -> tanh(+bias) -> score row chunks
            score_sb = smallp.tile([1, T], F32, tag="scorerow")
            for ch in range(NCH):
                psc = pscp.tile([1, 512], F32)
                for us_ in range(US):
                    ph1 = ph1p.tile([P, 512], F32)
                    for ds_ in range(DS):
                        nc.tensor.matmul(
                            ph1,
                            w1_sb[:, ds_, us_ * P:(us_ + 1) * P],
                            ft[:, ds_, ch * 512:(ch + 1) * 512],
                            start=(ds_ == 0),
                            stop=(ds_ == DS - 1),
                        )
                    th = tanhp.tile([P, 512], F32R)
                    nc.scalar.activation(
                        th, ph1, AF.Tanh, bias=bias_sb[:, us_, b:b + 1]
                    )
                    nc.tensor.matmul(
                        psc,
                        v_sb[:, us_:us_ + 1],
                        th,
                        start=(us_ == 0),
                        stop=(us_ == US - 1),
                    )
                nc.scalar.activation(
                    score_sb[:, ch * 512:(ch + 1) * 512], psc, AF.Copy
                )

            # score row -> columns (t on partitions), exp, sum, 1/sum
            pcols = pmiscp.tile([P, 16], F32, tag="pcols")
            for tt_ in range(TT):
                nc.tensor.transpose(
                    pcols[:, tt_:tt_ + 1],
                    score_sb[:, tt_ * P:(tt_ + 1) * P],
                    ident_f32[0:1, 0:1],
                )
            exp_cols = smallp.tile([P, TT], F32R, tag="expcols")
            nc.scalar.activation(exp_cols, pcols, AF.Exp)

            psum_t = pscp.tile([1, 512], F32, tag="psc")
            nc.tensor.matmul(
                psum_t[:, :TT], ones_col, exp_cols, start=True, stop=True
            )
            sum_sb = smallp.tile([1, 1], F32, tag="sums")
            nc.vector.tensor_reduce(
                sum_sb, psum_t[:, :TT], axis=mybir.AxisListType.X,
                op=mybir.AluOpType.add,
            )
            recip_sb = smallp.tile([1, 1], F32, tag="recip")
            nc.vector.reciprocal(recip_sb, sum_sb)

            # context = (exp_cols.T @ full) / sum
            pctx = pmiscp.tile([1, 512], F32, tag="pctx")
            for tt_ in range(TT):
                nc.tensor.matmul(
                    pctx,
                    exp_cols[:, tt_:tt_ + 1],
                    nat[:, tt_, :],
                    start=(tt_ == 0),
                    stop=(tt_ == TT - 1),
                )
            ctx_row = smallp.tile([1, D], F32, tag="ctxrow")
            nc.vector.tensor_scalar_mul(ctx_row, pctx, recip_sb)
            nc.sync.dma_start(ctx_d[b:b + 1], ctx_row)

    nc.compile()
    _CACHE["nc"] = nc
    return nc


def _runner():
    """Build (once) a cached jitted 8-core executor mirroring
    bass2jax.run_bass_via_pjrt, so repeat calls skip retracing."""
    if "runner" in _CACHE:
        return _CACHE["runner"]

    import jax
    import numpy as _np
    from jax.sharding import Mesh, PartitionSpec
    from jax.experimental.shard_map import shard_map

    import concourse.mybir as mybir
    from concourse import bass2jax

    bass2jax.install_neuronx_cc_hook()
    nc = _build()

    pid_name = nc.partition_id_tensor.name if nc.partition_id_tensor else None
    in_names, out_names, out_avals = [], [], []
    for alloc in nc.m.functions[0].allocations:
        if not isinstance(alloc, mybir.MemoryLocationSet):
            continue
        name = alloc.memorylocations[0].name
        if alloc.kind == "ExternalInput":
            if name != pid_name:
                in_names.append(name)
        elif alloc.kind == "ExternalOutput":
            out_names.append(name)
            out_avals.append(jax.core.ShapedArray(
                tuple(alloc.tensor_shape), mybir.dt.np(alloc.dtype)))
    n_params = len(in_names)
    all_names = in_names + out_names
    if pid_name is not None:
        all_names = all_names + [pid_name]

    def _body(*args):
        operands = list(args)
        if pid_name is not None:
            operands.append(bass2jax.partition_id_tensor())
        outs = bass2jax._bass_exec_p.bind(
            *operands,
            out_avals=tuple(out_avals),
            in_names=tuple(all_names),
            out_names=tuple(out_names),
            lowering_input_output_aliases=(),
            sim_require_finite=True,
            sim_require_nnan=True,
            nc=nc,
        )
        return tuple(outs)

    devices = jax.devices()[:NCORES]
    mesh = Mesh(_np.asarray(devices), ("core",))
    n_outs = len(out_names)
    in_specs = (PartitionSpec("core"),) * (n_params + n_outs)
    out_specs = (PartitionSpec("core"),) * n_outs
    fn = jax.jit(
        shard_map(_body, mesh=mesh, in_specs=in_specs, out_specs=out_specs,
                  check_rep=False),
        keep_unused=True,
    )
    out_zero_shapes = [
        (NCORES * a.shape[0],) + tuple(a.shape[1:]) for a in out_avals
    ]
    _CACHE["runner"] = (fn, in_names, out_names, out_avals, out_zero_shapes)
    return _CACHE["runner"]


def _concat_inputs(full, last, W1, b1, W2, b2, V):
    full = np.ascontiguousarray(np.asarray(full, np.float32))
    last = np.ascontiguousarray(np.asarray(last, np.float32))
    params = {
        "W1": np.ascontiguousarray(np.asarray(W1, np.float32)),
        "b1": np.ascontiguousarray(np.asarray(b1, np.float32)),
        "W2": np.ascontiguousarray(np.asarray(W2, np.float32)),
        "b2": np.ascontiguousarray(np.asarray(b2, np.float32)),
        "V": np.ascontiguousarray(np.asarray(V, np.float32)),
    }
    per_core_data = {"full": full, "last": last}
    _, in_names, _, _, _ = _runner()
    concat = []
    for name in in_names:
        if name in per_core_data:
            concat.append(per_core_data[name])  # axis0 = B = NCORES*BL
        else:
            p = params[name]
            concat.append(np.concatenate([p] * NCORES, axis=0))
    return concat


def kernel(full, last, W1, b1, W2, b2, V, bV, **_unused):
    fn, in_names, out_names, out_avals, out_zero_shapes = _runner()
    concat = _concat_inputs(full, last, W1, b1, W2, b2, V)
    zeros = [np.zeros(s, np.float32) for s in out_zero_shapes]
    outs = fn(*concat, *zeros)
    out = np.asarray(outs[0])  # [B, D]
    return out.astype(np.float32)


def bench(full, last, W1, b1, W2, b2, V, bV=None, iters=20, **_unused):
    """Steady-state per-call time with device-resident inputs (seconds)."""
    import time as _time

    import jax

    fn, in_names, out_names, out_avals, out_zero_shapes = _runner()
    concat = _concat_inputs(full, last, W1, b1, W2, b2, V)
    zeros = [np.zeros(s, np.float32) for s in out_zero_shapes]
    dev_in = [jax.device_put(a) for a in concat]
    dev_zero = [jax.device_put(z) for z in zeros]
    r = fn(*dev_in, *dev_zero)
    jax.block_until_ready(r)
    t0 = _time.time()
    for _ in range(iters):
        r = fn(*dev_in, *dev_zero)
    jax.block_until_ready(r)
    return (_time.time() - t0) / iters
